# revision 13
# baseline (speedup 1.0000x reference)
"""Boolean OR-matmul kernel for Trainium2 (8 NeuronCores).

out[b, i] = OR_j (x[b, j] AND w[i, j])  ==  (x_f32 @ w.T_f32) > 0

Strategy:
- Shard bit_weights rows (layer_size 8192) across 8 cores -> 1024 rows/core,
  replicate x. No cross-core reduction needed; host concatenates column
  blocks of the output.
- Encode bools as fp8_e4m3 0.0/1.0 (bit pattern 0x38 == 1.0). Products are
  exactly 0/1, PSUM accumulates fp32 (counts <= 8192 < 2^24, exact), so
  (count > 0) is exact.
- Host pre-transposes both operands to put the contraction dim (in_features
  D) on the SBUF partition axis: xT (D, B), wT (D, Lshard). This makes every
  DMA a clean 2D/3D strided pattern with >=512B contiguous runs.
- PE does fp8 DoubleRow matmuls (K=256 per instruction), k-innermost per
  PSUM tile so the accumulation group stays dense and HAM stays warm.
- DVE thresholds PSUM fp32 -> uint8 0/1 via is_gt, DMA out.
"""

import sys

for _p in ("/opt/trn_rl_repo",):
    if _p not in sys.path:
        sys.path.insert(0, _p)

import numpy as np
import ml_dtypes

import concourse.bass as bass
import concourse.tile as tile
from concourse import bacc, mybir
from concourse.bass_utils import run_bass_kernel_spmd

P = 128          # SBUF partitions / PE contraction per k-subtile
N_CORES = 8

# Full problem shapes (hardcoded per harness contract)
BATCH = 4096
IN_DIM = 8192
LAYER_SIZE = 8192
L_SHARD = LAYER_SIZE // N_CORES  # 1024


def build_nc(B, D, L, b_slab=512, n_free=512, use_dr=True):
    """Build the per-core Bass program.

    Per-core inputs : xT (D, B) fp8e4, wT (D, L) fp8e4
    Per-core output : out (B, L) uint8 (0/1)
    """
    assert D % (2 * P) == 0 and B % P == 0
    assert L % n_free == 0
    KSUB = D // P               # k-subtiles of 128
    NL = L // n_free            # l tiles
    assert B % b_slab == 0
    slabs = [b_slab] * (B // b_slab)
    offsets = [sum(slabs[:i]) for i in range(len(slabs))]

    nc = bacc.Bacc(None, target_bir_lowering=False, debug=False)
    xT = nc.dram_tensor("xT", [D, B], mybir.dt.float8e4, kind="ExternalInput")
    wT = nc.dram_tensor("wT", [D, L], mybir.dt.float8e4, kind="ExternalInput")
    out = nc.dram_tensor("out", [B, L], mybir.dt.uint8, kind="ExternalOutput")

    xT_r = xT.rearrange("(nk p) b -> p nk b", p=P)   # [128, KSUB, B]
    wT_r = wT.rearrange("(nk p) l -> p nk l", p=P)   # [128, KSUB, L]

    with tile.TileContext(nc) as tc:
        # Chunked tiles: separate tile objects give chunk-granular DMA->MM
        # dependencies, so the first matmuls start as soon as the leading
        # chunks arrive instead of waiting out the full 12 MB preload
        # (50 us PE-idle unchunked). Graduated sizes: tiny leading chunks
        # minimize the first-matmul gate, larger trailing chunks keep the
        # DMA count low.
        bounds = sorted({b for b in (0, 2, 4, 8, 16, 32, 48) if b < KSUB} | {KSUB})
        chunks = list(zip(bounds[:-1], bounds[1:]))  # [(lo, hi), ...]
        ks2chunk = {}
        for ci, (lo, hi) in enumerate(chunks):
            for ks in range(lo, hi):
                ks2chunk[ks] = (ci, ks - lo)
        with (
            tc.tile_pool(name="wpool", bufs=1) as wpool,
            tc.tile_pool(name="xpool", bufs=2) as xpool,
            tc.tile_pool(name="opool", bufs=4) as opool,
            tc.tile_pool(name="psum", bufs=8, space="PSUM") as pspool,
        ):
            w_tiles = [
                wpool.tile([P, hi - lo, L], mybir.dt.float8e4, name=f"w{j}")
                for j, (lo, hi) in enumerate(chunks)
            ]

            for i, (b0, bs) in enumerate(zip(offsets, slabs)):
                MSUB = bs // P
                x_chunks = []
                for j, (lo, hi) in enumerate(chunks):
                    if i == 0:
                        # Interleave resident-weight loads with slab-0 x
                        # loads in k-consumption order so the PE starts
                        # as early as possible.
                        nc.sync.dma_start(
                            out=w_tiles[j][:], in_=wT_r[:, lo:hi, :]
                        )
                    xt = xpool.tile(
                        [P, hi - lo, bs], mybir.dt.float8e4,
                        tag=f"x{j}", name=f"x{j}",
                    )
                    nc.sync.dma_start(
                        out=xt[:], in_=xT_r[:, lo:hi, b0 : b0 + bs]
                    )
                    x_chunks.append(xt)

                kstep = 2 if use_dr else 1

                def mm(ps, m, l, ks):
                    ci, off = ks2chunk[ks]
                    xt, wt = x_chunks[ci], w_tiles[ci]
                    if use_dr:
                        lhsT = xt[:, off : off + 2, m * P : (m + 1) * P]
                        rhs = wt[:, off : off + 2, l * n_free : (l + 1) * n_free]
                    else:
                        lhsT = xt[:, off, m * P : (m + 1) * P]
                        rhs = wt[:, off, l * n_free : (l + 1) * n_free]
                    nc.tensor.matmul(
                        ps[:],
                        lhsT,
                        rhs,
                        start=(ks == 0),
                        stop=(ks == KSUB - kstep),
                        perf_mode=(
                            mybir.MatmulPerfMode.DoubleRow if use_dr else None
                        ),
                        skip_group_check=True,
                    )

                def drain(ps, m, l):
                    ob = opool.tile([P, n_free], mybir.dt.uint8, tag="ob", name="ob")
                    nc.vector.tensor_scalar(
                        out=ob[:],
                        in0=ps[:],
                        scalar1=0.0,
                        scalar2=None,
                        op0=mybir.AluOpType.is_gt,
                    )
                    nc.sync.dma_start(
                        out=out[b0 + m * P : b0 + (m + 1) * P,
                                l * n_free : (l + 1) * n_free],
                        in_=ob[:],
                    )

                groups = [(m, l) for m in range(MSUB) for l in range(NL)]
                if i == 0 and len(groups) <= 8:
                    # Slab 0 is DMA-paced (the W+X broadcast is still in
                    # flight): run k OUTERMOST across all groups, one PSUM
                    # bank each, so every arriving k-chunk feeds 8x more PE
                    # work and the PE never outruns the DMA wave.
                    pss = {
                        g: pspool.tile(
                            [P, n_free], mybir.dt.float32, tag="ps", name="ps"
                        )
                        for g in groups
                    }
                    for ks in range(0, KSUB, kstep):
                        for m, l in groups:
                            mm(pss[(m, l)], m, l, ks)
                    for m, l in groups:
                        drain(pss[(m, l)], m, l)
                else:
                    for m, l in groups:
                        ps = pspool.tile(
                            [P, n_free], mybir.dt.float32, tag="ps", name="ps"
                        )
                        for ks in range(0, KSUB, kstep):
                            mm(ps, m, l, ks)
                        drain(ps, m, l)
    nc.compile()
    return nc


def to_fp8_bits(bool_arr_T):
    """bool/uint8 0-1 array -> fp8_e4m3 bytes holding 0.0 / 1.0 (0x38)."""
    a = np.ascontiguousarray(bool_arr_T).view(np.uint8) * np.uint8(0x38)
    return a.view(ml_dtypes.float8_e4m3)


_NC_CACHE = {}


def _get_nc(B, D, L):
    key = (B, D, L)
    if key not in _NC_CACHE:
        _NC_CACHE[key] = build_nc(B, D, L)
    return _NC_CACHE[key]


def run_spmd(x, bit_weights, trace=False, B=BATCH, D=IN_DIM, L_total=LAYER_SIZE):
    """Shared runner: returns (full bool output, BassKernelResults)."""
    n = N_CORES
    L = L_total // n
    nc = _get_nc(B, D, L)

    xT = to_fp8_bits(x.view(np.uint8).T)                      # (D, B)
    w_u8 = bit_weights.view(np.uint8)
    in_maps = []
    for m in range(n):
        wT_m = to_fp8_bits(w_u8[m * L : (m + 1) * L, :].T)    # (D, L)
        in_maps.append({"xT": xT, "wT": wT_m})

    res = run_bass_kernel_spmd(nc, in_maps, core_ids=list(range(n)), trace=trace)
    full = np.concatenate([res.results[m]["out"] for m in range(n)], axis=1)
    return full.view(np.bool_), res


def kernel(x, bit_weights):
    full, _ = run_spmd(np.asarray(x), np.asarray(bit_weights))
    return full
